# revision 31
# baseline (speedup 1.0000x reference)
"""Trainium2 Bass kernel for a 3-layer difflogic network (nn_Net_48610439856713).

Math: each layer o computes softmax(w[o])·ops16(a, b) with a = h[:, ia[o]],
b = h[:, ib[o]].  The 16 relaxed logic gates are all affine in {1, a, b, ab},
so the layer reduces to  h' = C0 + C1·a + C2·b + C3·a·b  with 4 per-neuron
coefficients derived on-device from softmax(w).

Sharding: 2 batch groups x 4 neuron shards over the 8 cores.  Core c handles
batch rows [(c//4)*256, ...) and neuron shard c%4 of every layer.  Activations
are bf16 in a transposed packed layout h^T[neuron, batch]; each layer's shard
outputs are exchanged with a 4-rank AllGather so every core holds the full
previous layer as its gather source.  Gathers use the SWDGE dma_gather
primitive (cost ~8ns/index of Q7 descriptor generation, the kernel's
bottleneck — which is why indices per core are minimized via neuron sharding).

Host-side bookkeeping is integer/layout only: slot permutations, index
relabeling through the packed layout, int16 index wrapping, weight-row
packing.  All float arithmetic (softmax, combine, sums) runs on device.
"""

import os
import numpy as np

P = 128
B = 512
BG = 2                  # batch groups
SH = 4                  # neuron shards
BC = B // BG            # 256 batch per core
IN = 193
NGROUP = 3
TAU = 100.0
N_CORES = 8

# layers 1/2: 16000 real neurons -> 4096 slots/shard (96 pads each)
NJ12 = 32               # j-columns per shard
REAL12 = 4000           # real neurons per shard
NS12 = NJ12 * P         # 4096 slots per shard
# layer 3: 15999 real -> 33 j-cols/shard; group g owns local j in [11g, 11g+11)
NJ3 = 33
JPG = 11                # j-cols per group per shard
NS3 = NJ3 * P           # 4224 slots per shard
SPG = 15999 // NGROUP   # 5333 real slots per group

_CACHE = {}


def _build_nc():
    import concourse.bacc as bacc
    import concourse.tile as tile
    import concourse.mybir as mybir

    f32 = mybir.dt.float32
    bf16 = mybir.dt.bfloat16
    i16 = mybir.dt.int16
    Alu = mybir.AluOpType
    Act = mybir.ActivationFunctionType
    Ax = mybir.AxisListType

    nc = bacc.Bacc("TRN2", target_bir_lowering=False, debug=False, num_devices=N_CORES)

    # ---- I/O ----
    xT = nc.dram_tensor("xT", [IN, BC], f32, kind="ExternalInput")
    wps = [
        nc.dram_tensor("w1p", [P, NJ12 * 16], f32, kind="ExternalInput"),
        nc.dram_tensor("w2p", [P, NJ12 * 16], f32, kind="ExternalInput"),
        nc.dram_tensor("w3p", [P, NJ3 * 16], f32, kind="ExternalInput"),
    ]
    idxs = []
    for l, ns in ((1, NS12), (2, NS12), (3, NS3)):
        # combined a+b index stream, chunk-interleaved: [a-chunk0 b-chunk0 ...]
        idxs.append(
            nc.dram_tensor(f"i{l}", [P, 2 * ns // 16], i16, kind="ExternalInput")
        )
    out_d = nc.dram_tensor("out", [1, NGROUP * BC], f32, kind="ExternalOutput")

    # collective buffers (h exchange, NCH j-chunks pipelined) and partial-sum
    # exchange.  g layout is chunk-major: row r = k*SH*P + s*P + p, unit
    # r*JCH + (j % JCH)  with JCH = NJ12//NCH j-cols per chunk.
    NCH_ = NCH
    JCH_ = JCH
    cins = [
        [
            nc.dram_tensor(f"cin{l}_{k}", [P, JCH * BC], bf16, kind="Internal")
            for k in range(NCH)
        ]
        for l in (1, 2)
    ]
    gs_ = [
        nc.dram_tensor("g1", [NCH * SH * P, JCH * BC], bf16, kind="Internal"),
        nc.dram_tensor("g2", [NCH * SH * P, JCH * BC], bf16, kind="Internal"),
    ]
    # warm-up collective: absorbs first-collective firmware latency while the
    # layer-1 gathers run.  Output is an (ignored) ExternalOutput so DCE keeps it.
    win = nc.dram_tensor("win", [P, 16], f32, kind="Internal")
    warm = nc.dram_tensor("warm", [SH * P, 16], f32, kind="Internal")
    pin = nc.dram_tensor("pin", [1, NGROUP * BC], f32, kind="Internal")
    pall = nc.dram_tensor("pall", [SH, NGROUP * BC], f32, kind="Internal")

    shard_groups = [[0, 1, 2, 3], [4, 5, 6, 7]]

    with tile.TileContext(nc) as tc:
        with (
            tc.tile_pool(name="big", bufs=1) as big,
            tc.tile_pool(name="prep", bufs=2) as prep,
            tc.tile_pool(name="small", bufs=2) as small,
            tc.tile_pool(name="psum", bufs=1, space="PSUM") as psum,
        ):
            layers = [
                (NJ12, NS12, f32, xT[:], idxs[0], wps[0], cins[0], gs_[0]),
                (
                    NJ12, NS12, bf16,
                    gs_[0][:].rearrange("r (j b) -> (r j) b", b=BC),
                    idxs[1], wps[1], cins[1], gs_[1],
                ),
                (
                    NJ3, NS3, bf16,
                    gs_[1][:].rearrange("r (j b) -> (r j) b", b=BC),
                    idxs[2], wps[2], None, None,
                ),
            ]  # cin entries are per-chunk lists for layers 1-2

            wsb = prep.tile([P, 16], f32, tag="wsb")
            nc.vector.memset(wsb[:], 0.0)
            nc.sync.dma_start(win[:], wsb[:])
            nc.gpsimd.collective_compute(
                "AllGather", Alu.bypass, replica_groups=shard_groups,
                ins=[win[:]], outs=[warm[:]],
            )

            h_final = None
            for li, (NJ, NS, gdt, src, iad, wp, cin, gout) in enumerate(layers):
                last = li == 2
                # ---- idx load first (gathers are the critical chain) ----
                iab = prep.tile([P, 2 * NS // 16], i16, tag="iab")
                nc.sync.dma_start(iab[:], iad[:])
                # ---- coefficient prep: C0..C3 [P, NJ] f32 ----
                wt = prep.tile([P, NJ * 16], f32, tag="wt")
                nc.sync.dma_start(wt[:], wp[:])
                e = prep.tile([P, NJ * 16], f32, tag="e")
                nc.scalar.activation(e[:], wt[:], Act.Exp)
                e3 = e[:].rearrange("p (j g) -> p j g", g=16)
                e4 = e[:].rearrange("p (j h q) -> p j h q", h=4, q=4)

                ssum = small.tile([P, NJ], f32, tag="ssum")
                nc.vector.reduce_sum(ssum[:], e3, axis=Ax.X)
                r = small.tile([P, NJ], f32, tag="r")
                nc.vector.reciprocal(r[:], ssum[:])

                c0 = small.tile([P, NJ], f32, tag="c0")
                c1 = small.tile([P, NJ], f32, tag="c1")
                c2 = small.tile([P, NJ], f32, tag="c2")
                c3 = small.tile([P, NJ], f32, tag="c3")

                nc.vector.reduce_sum(c0[:], e4[:, :, 2:4, :], axis=Ax.XY)
                t1 = small.tile([P, NJ], f32, tag="t1")
                t2 = small.tile([P, NJ], f32, tag="t2")
                nc.vector.reduce_sum(t1[:], e4[:, :, 0:2, 2:4], axis=Ax.XY)
                nc.vector.reduce_sum(t2[:], e4[:, :, 2:4, 0:2], axis=Ax.XY)
                nc.vector.tensor_sub(c1[:], t1[:], t2[:])
                t3 = small.tile([P, NJ], f32, tag="t3")
                t4 = small.tile([P, NJ], f32, tag="t4")
                nc.vector.reduce_sum(t3[:], e4[:, :, 1, :], axis=Ax.X)
                nc.vector.reduce_sum(t4[:], e4[:, :, 2, :], axis=Ax.X)
                nc.vector.tensor_sub(c2[:], t3[:], t4[:])
                f = small.tile([P, NJ, 7], f32, tag="f")
                nc.vector.tensor_sub(f[:], e3[:, :, 1:8], e3[:, :, 14:7:-1])
                u1 = small.tile([P, NJ], f32, tag="u1")
                u2 = small.tile([P, NJ], f32, tag="u2")
                nc.vector.tensor_sub(u1[:], f[:, :, 0], f[:, :, 1])
                nc.vector.tensor_add(u2[:], f[:, :, 3], f[:, :, 6])
                nc.vector.tensor_sub(u1[:], u1[:], u2[:])
                nc.vector.scalar_tensor_tensor(
                    c3[:], f[:, :, 5], -2.0, u1[:], op0=Alu.mult, op1=Alu.add
                )
                for ck in (c0, c1, c2, c3):
                    nc.vector.tensor_mul(ck[:], ck[:], r[:])



                # ---- chunked gathers + combine ----
                h = big.tile([P, NJ * BC], bf16, tag="h")
                h3 = h[:].rearrange("p (j b) -> p j b", b=BC)
                if last:
                    # group-aligned chunks so GroupSum reduces fire per chunk
                    chunks = [(0, 11), (11, 22), (22, NJ)]
                else:
                    chunks = [(k * JCH, (k + 1) * JCH) for k in range(NCH)]
                for ci, (j0, j1) in enumerate(chunks):
                    cw = j1 - j0
                    ab = big.tile([P, 2 * cw, BC], gdt, tag=f"ab{ci}")
                    nsc = 2 * cw * P
                    nc.gpsimd.dma_gather(
                        ab[:], src, iab[:, 2 * j0 * 8 : 2 * j1 * 8], nsc, nsc, BC,
                        single_packet=False,
                    )
                    tmp = big.tile([P, cw, BC], gdt, tag=f"t{ci}")
                    tmp2 = big.tile([P, cw, BC], gdt, tag=f"u{ci}")
                    for j in range(j0, j1):
                        jl = j - j0
                        aj = ab[:, jl]
                        bj = ab[:, cw + jl]
                        # h = a*(C3*b + C1) + (C2*b + C0); the two affine terms
                        # are single-src tensor_scalar ops (fast path), and no
                        # ACT op sits on the chunk-ship critical path.
                        nc.vector.tensor_scalar(
                            tmp[:, jl], bj, c3[:, j : j + 1], c1[:, j : j + 1],
                            op0=Alu.mult, op1=Alu.add,
                        )
                        nc.vector.tensor_scalar(
                            tmp2[:, jl], bj, c2[:, j : j + 1], c0[:, j : j + 1],
                            op0=Alu.mult, op1=Alu.add,
                        )
                        nc.vector.scalar_tensor_tensor(
                            tmp[:, jl], tmp[:, jl], 0.0, aj,
                            op0=Alu.bypass, op1=Alu.mult,
                        )
                        nc.vector.tensor_add(h3[:, j], tmp[:, jl], tmp2[:, jl])

                    if not last:
                        # ship this chunk as soon as it's combined
                        nc.sync.dma_start(
                            cin[ci][:], h[:, j0 * BC : j1 * BC]
                        )
                        nc.gpsimd.collective_compute(
                            "AllGather", Alu.bypass, replica_groups=shard_groups,
                            ins=[cin[ci][:]],
                            outs=[gout[ci * SH * P : (ci + 1) * SH * P, :]],
                        )
                if last:
                    h_final = h

            # ---- GroupSum: per-shard partials, then cross-shard AllGather+sum ----
            gs = prep.tile([P, NGROUP * BC], f32, tag="gs")
            for g in range(NGROUP):
                sl = h_final[:, g * JPG * BC : (g + 1) * JPG * BC].rearrange(
                    "p (j b) -> p b j", b=BC
                )
                nc.vector.reduce_sum(gs[:, g * BC : (g + 1) * BC], sl, axis=Ax.X)
            ones = prep.tile([P, 1], f32, tag="ones")
            nc.vector.memset(ones[:], 1.0)
            psc = prep.tile([1, NGROUP * BC], f32, tag="psc")
            HW = NGROUP * BC // 2
            for k in range(2):
                ps = psum.tile([1, HW], f32, tag=f"ps{k}")
                nc.tensor.matmul(
                    ps[:], ones[:], gs[:, k * HW : (k + 1) * HW],
                    start=True, stop=True,
                )
                nc.scalar.copy(psc[:, k * HW : (k + 1) * HW], ps[:])
            nc.sync.dma_start(pin[:], psc[:])
            nc.gpsimd.collective_compute(
                "AllGather", Alu.bypass, replica_groups=shard_groups,
                ins=[pin[:]], outs=[pall[:]],
            )
            pall_sb = prep.tile([SH, NGROUP * BC], f32, tag="pall_sb")
            nc.sync.dma_start(pall_sb[:], pall[:])
            ones4 = prep.tile([SH, 1], f32, tag="ones4")
            nc.vector.memset(ones4[:], 1.0)
            osb = prep.tile([1, NGROUP * BC], f32, tag="osb")
            for k in range(2):
                ps2 = psum.tile([1, HW], f32, tag=f"ps2{k}")
                nc.tensor.matmul(
                    ps2[:], ones4[:], pall_sb[:, k * HW : (k + 1) * HW],
                    start=True, stop=True,
                )
                nc.scalar.mul(osb[:, k * HW : (k + 1) * HW], ps2[:], 1.0 / TAU)
            # consume the warm-up collective's (all-zero) output so DCE keeps it
            wsb2 = prep.tile([1, 16], f32, tag="wsb2")
            nc.sync.dma_start(wsb2[:], warm[0:1, :])
            nc.vector.tensor_add(osb[:, :16], osb[:, :16], wsb2[:])
            nc.sync.dma_start(out_d[:], osb[:])

    nc.compile()
    return nc


def _wrap_idx(ii):
    w = ii.astype(np.int16).reshape(-1, 16).T
    return np.ascontiguousarray(np.tile(w, (8, 1)))


CHUNKS12 = [(0, 8), (8, 16), (16, 24), (24, 32)]
CHUNKS3 = [(0, 11), (11, 22), (22, 33)]


def _combine_idx(ia_eff, ib_eff, chunk_list):
    """Interleave a/b index streams per chunk: [a-chunk0, b-chunk0, a-chunk1, ...]"""
    parts = []
    for j0, j1 in chunk_list:
        parts.append(ia_eff[j0 * P : j1 * P])
        parts.append(ib_eff[j0 * P : j1 * P])
    return _wrap_idx(np.concatenate(parts))


def _pack_w(w_eff, nj):
    # local slot t = j*128 + p  ->  packed[p, j*16+g]
    return np.ascontiguousarray(
        w_eff.reshape(nj, P, 16).transpose(1, 0, 2).reshape(P, nj * 16)
    )


NCH = 4
JCH = NJ12 // NCH


def _src_unit12(i):
    """BC-row unit of layer-1/2 neuron i in the chunk-major AllGathered
    [NCH*SH*128, JCH*BC] layout: shard s = i//4000, local t = i - 4000s,
    p = t%128, j = t//128, chunk k = j//JCH; row = (k*SH+s)*128+p,
    unit = row*JCH + j%JCH."""
    s = i // REAL12
    t = i - s * REAL12
    p = t % P
    j = t // P
    k = j // JCH
    return ((k * SH + s) * P + p) * JCH + j % JCH


def _host_pack(inputs):
    x = np.asarray(inputs["x"], dtype=np.float32)
    w1 = np.asarray(inputs["w1"], dtype=np.float32)
    w2 = np.asarray(inputs["w2"], dtype=np.float32)
    w3 = np.asarray(inputs["w3"], dtype=np.float32)
    i1a = np.asarray(inputs["idx1a"]).astype(np.int64)
    i1b = np.asarray(inputs["idx1b"]).astype(np.int64)
    i2a = np.asarray(inputs["idx2a"]).astype(np.int64)
    i2b = np.asarray(inputs["idx2b"]).astype(np.int64)
    i3a = np.asarray(inputs["idx3a"]).astype(np.int64)
    i3b = np.asarray(inputs["idx3b"]).astype(np.int64)

    pad_row = np.full(16, -20.0, dtype=np.float32)
    pad_row[0] = 20.0  # softmax -> ~one-hot FALSE gate -> h = 0

    per_shard = [dict() for _ in range(SH)]
    # layers 1 and 2: shard s owns real neurons [s*4000, (s+1)*4000)
    for l, (w, ja, jb, srcf) in enumerate(
        (
            (w1, i1a, i1b, lambda i: i),
            (w2, i2a, i2b, _src_unit12),
        ),
        start=1,
    ):
        for s in range(SH):
            sel = slice(s * REAL12, (s + 1) * REAL12)
            w_eff = np.concatenate(
                [w[sel], np.tile(pad_row, (NS12 - REAL12, 1))], axis=0
            )
            ia_eff = np.zeros(NS12, dtype=np.int64)
            ib_eff = np.zeros(NS12, dtype=np.int64)
            ia_eff[:REAL12] = srcf(ja[sel])
            ib_eff[:REAL12] = srcf(jb[sel])
            per_shard[s][f"w{l}p"] = _pack_w(w_eff, NJ12)
            per_shard[s][f"i{l}"] = _combine_idx(ia_eff, ib_eff, CHUNKS12)

    # layer 3: group g's 5333 real neurons split over shards as
    # counts c_s = [1334, 1333, 1333, 1333]; within (s, g): local j in
    # [11g, 11g+11), rank m = (j-11g)*128 + p
    counts = np.array([1334, 1333, 1333, 1333])
    offs = np.concatenate([[0], np.cumsum(counts)[:-1]])
    u = np.arange(NS3)
    jj = u // P
    pp = u % P
    gg = jj // JPG
    m = (jj - gg * JPG) * P + pp
    for s in range(SH):
        real = m < counts[s]
        rid = gg * SPG + offs[s] + np.minimum(m, counts[s] - 1)
        w3_eff = w3[rid].copy()
        w3_eff[~real] = pad_row
        i3a_eff = np.where(real, _src_unit12(i3a[rid]), 0)
        i3b_eff = np.where(real, _src_unit12(i3b[rid]), 0)
        per_shard[s]["w3p"] = _pack_w(w3_eff, NJ3)
        per_shard[s]["i3"] = _combine_idx(i3a_eff, i3b_eff, CHUNKS3)

    in_maps = []
    for c in range(N_CORES):
        G, s = c // SH, c % SH
        m_ = dict(per_shard[s])
        m_["xT"] = np.ascontiguousarray(x[G * BC : (G + 1) * BC].T)
        in_maps.append(m_)
    return in_maps


LAST_RESULTS = None


def kernel(**inputs):
    global LAST_RESULTS
    from concourse.bass_utils import run_bass_kernel_spmd

    if "nc" not in _CACHE:
        _CACHE["nc"] = _build_nc()
    nc = _CACHE["nc"]

    in_maps = _host_pack(inputs)
    trace = bool(int(os.environ.get("KERNEL_TRACE", "0")))
    res = run_bass_kernel_spmd(
        nc, in_maps, core_ids=list(range(N_CORES)), trace=trace
    )
    LAST_RESULTS = res

    out = np.empty((B, NGROUP), dtype=np.float32)
    for g_ in range(BG):
        rc = res.results[g_ * SH]["out"].reshape(NGROUP, BC)
        out[g_ * BC : (g_ + 1) * BC, :] = rc.T
    return out
